# revision 2
# baseline (speedup 1.0000x reference)
"""Trainium2 Bass kernel for the 4-modality attention-fusion module.

Computes, for full inputs mod0..mod3 [16384, 1024] f32 and W [1024, 1024] f32:
    scores_m = mod_m @ W.T                      (per modality)
    attn     = softmax over m of scores         (elementwise over [B, L])
    fused    = sum_m mod_m * attn_m
    scaler_b = 1 + #{m : sum_l mod_m[b, l] == 0}
    out      = fused * scaler[:, None]

Sharded data-parallel over 8 NeuronCores along batch (2048 rows each), W
replicated. Design (validated against the CoreSim cost model, ~190us/core
simulated vs ~345us for the f32r baseline):
  - scores via fp8(e4m3) DoubleRow matmuls (0.5 cyc/row, 2x contraction per
    instruction = 4x the f32r rate) with a 3-term hi/lo error split:
    x@W ~= xh@Wh + xh@Wl + xl@Wh, where xh = fp8(x), xl = fp8(x - xh), and
    (Wh, Wl) is the host-precomputed fp8 hi/lo split of 16*W (the 16x
    prescale keeps fp8 W values in the normal range; it is folded back out
    through the exp's free scale parameter). Measured 6.5e-3 max-rel error
    vs the 2e-2 gate.
  - input tiles are cast f32->bf16 with accum_out carrying the exact f32
    row sums (zero-modality detection is free);
  - bf16 transposes ride the DMA xbar (dma_start_transpose, no PE time);
    the fp8 stationaries derive from the transposed copy on Pool;
  - h-major accumulation chains: each k-half's exps/softmax-tail runs while
    the other half's matmuls occupy the PE (hides startup and drain);
  - softmax tail in bf16 at DVE 2x, denominator partials on Pool;
  - five-queue balance: SP = transpose DMAs + 3 input DMAs + output halves,
    ACT = exps + 3 casts + 1 input DMA, DVE = 1 cast + tail, Pool = fp8
    derivation + W loads + denominator partials, PE = matmuls only.
"""

import sys

sys.path.insert(0, "/opt/trn_rl_repo")

from contextlib import ExitStack

import numpy as np

import concourse.bass as bass
import concourse.bacc as bacc
import concourse.mybir as mybir
import concourse.tile as tile
from concourse.bass_utils import run_bass_kernel_spmd
from concourse.masks import make_identity

F32 = mybir.dt.float32
BF16 = mybir.dt.bfloat16
FP8 = mybir.dt.float8e4
AF = mybir.ActivationFunctionType
DR = mybir.MatmulPerfMode.DoubleRow

SW = 16.0            # W prescale so fp8(e4m3) W values sit in the normal range
ISW = 1.0 / SW       # folded back out via the exp's free scale parameter

N_CORES = 8
B_FULL = 16384
L = 1024
P = 128
B_SHARD = B_FULL // N_CORES          # 2048
NPT = B_SHARD // P                   # 16 patient tiles per core
NM = 4                               # modalities
NLC = L // P                         # 8 l-chunks (contraction)
NH = 2                               # PSUM k halves
KH = L // NH                         # 512

_CACHE: dict = {}


def _build(repeat: int = 1):
    nc = bacc.Bacc("TRN2", target_bir_lowering=False, debug=False)
    mods_d = [
        nc.dram_tensor(f"mod{m}", [B_SHARD, L], F32, kind="ExternalInput").ap()
        for m in range(NM)
    ]
    # W arrives pre-split on the host into the fp8 hi/lo DoubleRow moving
    # layout: whT[p, lc, k] = fp8(SW * W[k, lc*128+p]), wlT = fp8 residual.
    wh_d = nc.dram_tensor("whT", [P, NLC, L], FP8, kind="ExternalInput").ap()
    wl_d = nc.dram_tensor("wlT", [P, NLC, L], FP8, kind="ExternalInput").ap()
    out_d = nc.dram_tensor("out", [B_SHARD, L], F32, kind="ExternalOutput").ap()

    with tile.TileContext(nc) as tc, ExitStack() as ctx:
        const_p = ctx.enter_context(tc.tile_pool(name="const", bufs=1))
        wt_p = ctx.enter_context(tc.tile_pool(name="wt", bufs=1))
        wload_p = ctx.enter_context(tc.tile_pool(name="wload", bufs=1))
        wkb_p = ctx.enter_context(tc.tile_pool(name="wkb", bufs=2))
        mod_p = ctx.enter_context(tc.tile_pool(name="mod", bufs=3))
        modb_p = ctx.enter_context(tc.tile_pool(name="modb", bufs=4))
        modt_p = ctx.enter_context(tc.tile_pool(name="modt", bufs=2))
        modtb_p = ctx.enter_context(tc.tile_pool(name="modtb", bufs=2))
        e_p = ctx.enter_context(tc.tile_pool(name="e", bufs=2))
        nm_p = ctx.enter_context(tc.tile_pool(name="nm", bufs=2))
        den_p = ctx.enter_context(tc.tile_pool(name="den", bufs=1))
        tmp_p = ctx.enter_context(tc.tile_pool(name="tmp", bufs=4))
        out_p = ctx.enter_context(tc.tile_pool(name="outp", bufs=2))
        ps_q = ctx.enter_context(
            tc.tile_pool(name="ps_q", bufs=7, space=bass.MemorySpace.PSUM)
        )

        identf = const_p.tile([P, P], F32, tag="identf")
        make_identity(nc, identf[:])
        identb = const_p.tile([P, P], BF16, tag="identb")
        nc.vector.tensor_copy(identb[:], identf[:])

        # ---- W resident in SBUF as the host-prepared fp8 hi/lo split:
        # two 1MB DMAs on separate queues, nothing else to do on-device.
        wh = wt_p.tile([P, NLC, L], FP8, tag="wh")
        wl = wt_p.tile([P, NLC, L], FP8, tag="wl")
        nc.gpsimd.dma_start(wl[:], wl_d[:, :, :])
        nc.gpsimd.dma_start(wh[:], wh_d[:, :, :])

        # ---------------- main loop, software-pipelined ----------------
        rep_cm = (
            tc.For_i(
                0,
                repeat,
                1,
                hint_engines=(
                    mybir.EngineType.PE,
                    mybir.EngineType.DVE,
                    mybir.EngineType.Activation,
                    mybir.EngineType.Pool,
                    mybir.EngineType.SP,
                ),
            )
            if repeat > 1
            else None
        )
        if rep_cm is not None:
            rep_cm.__enter__()

        def new_bundle(p):
            row = slice(p * P, (p + 1) * P)
            mods = []
            for m in range(NM):
                mt = mod_p.tile([P, L], F32, tag=f"mod{m}")
                mods.append(mt)
            psums = tmp_p.tile([P, NM], F32, tag="psums")
            return {"p": p, "row": row, "mods": mods, "psums": psums,
                    "mbs": {}, "xhs": {}, "xls": {}, "es": {}, "tl": {}}

        def emit_mod_dmas(b, first=False):
            # mod3 rides the ACT queue (emitted at its segment-queue head by
            # the caller when first=False), mod0..2 ride SP
            for m in range(3):
                nc.sync.dma_start(b["mods"][m][:], mods_d[m][b["row"], :])
            if first:
                nc.scalar.dma_start(b["mods"][3][:], mods_d[3][b["row"], :])

        def emit_mod3_dma(b):
            nc.scalar.dma_start(b["mods"][3][:], mods_d[3][b["row"], :])

        def emit_cast(b, m):
            """f32->bf16 cast on ACT; accum_out gives the exact f32 row sum.
            Modalities are cast pairwise into one [P, 2L] tile so a single
            xbar transpose DMA covers two modalities."""
            j, half = divmod(m, 2)
            if half == 0:
                mb = modb_p.tile([P, 2 * L], BF16, tag=f"mb{j}")
                b["mbs"][j] = mb
            mb = b["mbs"][j]
            if m == 0:
                # mod0's cast rides DVE to keep ACT under the PE pace
                nc.vector.tensor_scalar(
                    out=mb[:, 0:L],
                    in0=b["mods"][0][:],
                    scalar1=0.0,
                    scalar2=None,
                    op0=mybir.AluOpType.add,
                    op1=mybir.AluOpType.add,
                    accum_out=b["psums"][:, 0:1],
                )
            else:
                nc.scalar.activation(
                    mb[:, half * L : (half + 1) * L],
                    b["mods"][m][:],
                    AF.Copy,
                    accum_out=b["psums"][:, m : m + 1],
                )

        def emit_tdma_pair(b, j):
            """Transpose modality pair j via the DMA xbar (no PE), then the
            two fp8 DoubleRow stationaries on Pool: xh = fp8(xT),
            xl = fp8(xT - xh)."""
            mT = modtb_p.tile([P, 2 * NLC, P], BF16, tag=f"mT{j}")
            nc.sync.dma_start_transpose(mT[:], b["mbs"][j][:])
            xh = modt_p.tile([P, 2 * NLC, P], FP8, tag=f"xh{j}")
            nc.gpsimd.tensor_copy(xh[:], mT[:])
            xl = modt_p.tile([P, 2 * NLC, P], FP8, tag=f"xl{j}")
            nc.gpsimd.tensor_sub(xl[:], mT[:], xh[:])
            b["xhs"][j] = xh
            b["xls"][j] = xl

        def emit_mains_h(b, m, h):
            """One k-half accumulation chain for modality m: all 3 fp8
            DoubleRow terms over the 4 lc-pairs. h-major emission means the
            h0 chains only need the first half of Wh/Wl (startup) and the h0
            exps/tails run while the h1 chains occupy the PE (drain)."""
            sq = ps_q.tile([P, KH], F32, tag="sq")
            j, half = divmod(m, 2)
            xh, xl = b["xhs"][j], b["xls"][j]
            off = half * NLC
            terms = [(xh, wh), (xh, wl), (xl, wh)]
            for t, (st, mv) in enumerate(terms):
                for g in range(NLC // 2):
                    lc = 2 * g
                    nc.tensor.matmul(
                        sq[:],
                        st[:, off + lc : off + lc + 2, :],
                        mv[:, lc : lc + 2, h * KH : (h + 1) * KH],
                        start=(t == 0 and g == 0),
                        stop=(t == 2 and g == 3),
                        perf_mode=DR,
                    )
            return sq

        def emit_exp(b, m, h, sq):
            if m not in b["es"]:
                e = e_p.tile([P, L], BF16, tag=f"e{m}")
                b["es"][m] = e
            nc.scalar.activation(
                b["es"][m][:, h * KH : (h + 1) * KH], sq[:], AF.Exp, scale=ISW
            )

        def emit_segment(b, b1, b2):
            """Tile b's matmuls/exps; transpose stage for b1 (next tile);
            loads + casts for b2 (tile after next). Queue placement: the two
            transpose-DMA seqs go first on SP so their xbar transfers finish
            mid-segment, Pool derives fp8 stationaries right behind them."""
            if b2 is not None:
                emit_mod3_dma(b2)
            if b1 is not None:
                emit_tdma_pair(b1, 0)
                emit_tdma_pair(b1, 1)
            if b2 is not None:
                emit_mod_dmas(b2)
                emit_cast(b2, 0)
            for m in range(NM):
                sq = emit_mains_h(b, m, 0)
                emit_exp(b, m, 0, sq)
                if b2 is not None and m >= 1:
                    emit_cast(b2, m)
            for m in range(NM):
                sq = emit_mains_h(b, m, 1)
                emit_exp(b, m, 1, sq)

        def emit_tail_half(b, h):
            """Softmax fusion chain for one k-half of tile p. The h0 half is
            emitted inside segment p (its exps land mid-segment); the h1 half
            during segment p+1. d01/d23 ride Pool, the rest DVE."""
            tl = b["tl"]
            es, mbs, psums = b["es"], b["mbs"], b["psums"]
            e0, e1, e2, e3 = (es[m] for m in range(NM))
            hs = slice(h * KH, (h + 1) * KH)

            if h == 0:
                zt = tmp_p.tile([P, NM], F32, tag="zt")
                zs = tmp_p.tile([P, 1], F32, tag="zs")
                nc.vector.tensor_scalar(
                    out=zt[:],
                    in0=psums[:],
                    scalar1=0.0,
                    scalar2=None,
                    op0=mybir.AluOpType.is_equal,
                    op1=mybir.AluOpType.add,
                    accum_out=zs[:],
                )
                scaler = tmp_p.tile([P, 1], F32, tag="scaler")
                nc.vector.tensor_scalar_add(scaler[:], zs[:], 1.0)
                d01 = den_p.tile([P, L], F32, tag="d01")
                d23 = den_p.tile([P, L], F32, tag="d23")
                nms = []
                for m in range(NM):
                    nmt = nm_p.tile([P, L], BF16, tag=f"nm{m}")
                    nms.append(nmt)
                ot = out_p.tile([P, L], F32, tag="ot")
                tl["scaler"] = scaler
                tl["d01"] = d01
                tl["d23"] = d23
                tl["nms"] = nms
                tl["ot"] = ot
            scaler, d01, d23, nms, ot = (
                tl["scaler"], tl["d01"], tl["d23"], tl["nms"], tl["ot"]
            )

            nc.gpsimd.tensor_add(d01[:, hs], e0[:, hs], e1[:, hs])
            nc.gpsimd.tensor_add(d23[:, hs], e2[:, hs], e3[:, hs])

            for m in range(NM):
                j, half = divmod(m, 2)
                nc.vector.tensor_mul(
                    nms[m][:, hs],
                    es[m][:, hs],
                    mbs[j][:, half * L + h * KH : half * L + (h + 1) * KH],
                )
            nc.vector.tensor_add(d01[:, hs], d01[:, hs], d23[:, hs])
            nc.vector.tensor_add(nms[0][:, hs], nms[0][:, hs], nms[1][:, hs])
            nc.vector.tensor_add(nms[2][:, hs], nms[2][:, hs], nms[3][:, hs])
            nc.vector.tensor_add(nms[0][:, hs], nms[0][:, hs], nms[2][:, hs])

            nc.vector.reciprocal_approx_fast(out=d01[:, hs], in_=d01[:, hs])
            nc.vector.scalar_tensor_tensor(
                out=ot[:, hs],
                in0=d01[:, hs],
                scalar=scaler[:],
                in1=nms[0][:, hs],
                op0=mybir.AluOpType.mult,
                op1=mybir.AluOpType.mult,
            )
            nc.sync.dma_start(out_d[b["row"], hs], ot[:, hs])

        bundles = []
        b0 = new_bundle(0)
        emit_mod_dmas(b0, first=True)
        for m in range(NM):
            emit_cast(b0, m)
        emit_tdma_pair(b0, 0)
        emit_tdma_pair(b0, 1)
        b1_ = new_bundle(1)
        emit_mod_dmas(b1_)
        emit_mod3_dma(b1_)
        for m in range(NM):
            emit_cast(b1_, m)
        bundles = [b0, b1_]
        prev = None
        for p in range(NPT):
            b = bundles[p]
            b1 = bundles[p + 1] if p + 1 < NPT else None
            if p + 2 < NPT:
                b2 = new_bundle(p + 2)
                bundles.append(b2)
            else:
                b2 = None
            emit_segment(b, b1, b2)
            if prev is not None:
                emit_tail_half(prev, 1)
            emit_tail_half(b, 0)
            prev = b
        emit_tail_half(prev, 1)

        if rep_cm is not None:
            rep_cm.__exit__(None, None, None)

    nc.compile()
    return nc


def _get_nc(repeat: int = 1):
    key = ("nc", repeat)
    if key not in _CACHE:
        _CACHE[key] = _build(repeat)
    return _CACHE[key]


def host_w_split(w: np.ndarray) -> dict:
    """fp8 hi/lo split of W in the transposed DoubleRow moving layout."""
    import ml_dtypes

    f8 = ml_dtypes.float8_e4m3
    a16 = np.asarray(w, dtype=np.float32) * SW          # [k, l]
    wh = a16.astype(f8)
    rw = a16 - wh.astype(np.float32)
    wlo = rw.astype(f8)

    def to_t(a):  # [k, l] -> [p, lc, k]
        return np.ascontiguousarray(
            np.ascontiguousarray(a.T).reshape(NLC, P, L).transpose(1, 0, 2)
        )

    return {"whT": to_t(wh), "wlT": to_t(wlo)}


def _run(inputs, trace=False):
    nc = _get_nc()
    wsplit = host_w_split(inputs["W"])
    in_maps = []
    for c in range(N_CORES):
        sl = slice(c * B_SHARD, (c + 1) * B_SHARD)
        im = dict(wsplit)
        for m in range(NM):
            im[f"mod{m}"] = np.ascontiguousarray(
                np.asarray(inputs[f"mod{m}"], dtype=np.float32)[sl]
            )
        in_maps.append(im)
    return run_bass_kernel_spmd(
        nc, in_maps, core_ids=list(range(N_CORES)), trace=trace
    )


def kernel(**inputs) -> np.ndarray:
    res = _run(inputs, trace=False)
    return np.concatenate(
        [res.results[c]["out"] for c in range(N_CORES)], axis=0
    ).astype(np.float32)
